# revision 1
# baseline (speedup 1.0000x reference)
"""JointNet (RNN-T) Bass kernel for trn2, 8 NeuronCores.

Math (per reference):
  he = enc @ W1[:D]           (B,T,H)
  hd = dec @ W1[D:]           (B,U,H)
  h  = gelu_tanh(he[:,:,None] + hd[:,None,:] + b1)    (B,T,U,H)
  out = h @ W2                (B,T,U,V)

Sharding: flatten (B,T) -> 1024 rows; core k takes rows [k*128,(k+1)*128)
(= batch b=k//2, t-range (k%2)*128..+128). W1/b1/W2 replicated.

Per-core device layout: everything transposed so H lives on partitions:
  heT[hc]  [128, T=128]   = We^T @ encT  (+ b1)       (4 H-chunks)
  hdT[hc]  [128, U=96]    = Wd^T @ decT
  hT       [128, (t,u)]   = gelu(hdT bcast + heT bcast)   (bf16)
  out_psum [128 pairs, 512] = hT.T @ W2  -> SBUF -> DRAM (contiguous rows)
"""

import os
import numpy as np
import ml_dtypes

B, T, U, D, H, V = 4, 256, 96, 512, 512, 1024
NCORES = 8
TSH = (B * T) // NCORES          # 128 (b,t) rows per core
PAIRS = TSH * U                  # 12288 output rows per core
P = 128                          # partitions
DC = D // P                      # 4 contraction chunks for W1 matmuls
HC = H // P                      # 4 H chunks
TB = 16                          # t-block size for broadcast/gelu staging
NTB = TSH // TB                  # 8 t-blocks
BLK = (TB * U) // P              # 12 pair-blocks of 128 per t-block

TRACE = False                    # test.py flips this to profile
OUT_BF16 = True                  # stage output in bf16 (halves store DMA)
LAST_RESULT = None               # BassKernelResults stash for test.py

_NC_CACHE = {}


def _build_module(mm_bf16=True, out_bf16=OUT_BF16):
    import concourse.bass as bass
    import concourse.mybir as mybir
    import concourse.tile as tile
    from concourse import bacc

    f32 = mybir.dt.float32
    bf16 = mybir.dt.bfloat16
    mmdt = bf16 if mm_bf16 else mybir.dt.float32

    nc = bacc.Bacc("TRN2", target_bir_lowering=False, debug=False)
    encT = nc.dram_tensor("encT", [D, TSH], bf16, kind="ExternalInput")
    decT = nc.dram_tensor("decT", [D, U], bf16, kind="ExternalInput")
    w1 = nc.dram_tensor("W1", [2 * D, H], bf16, kind="ExternalInput")
    b1pc = nc.dram_tensor("b1pc", [P, HC], f32, kind="ExternalInput")
    w2 = nc.dram_tensor("W2c", [H, V], mmdt, kind="ExternalInput")
    odt = bf16 if out_bf16 else f32
    out = nc.dram_tensor("out", [PAIRS, V], odt, kind="ExternalOutput")

    with tile.TileContext(nc) as tc:
        with (
            tc.tile_pool(name="const", bufs=1) as const,
            tc.tile_pool(name="sums", bufs=3) as sums,
            tc.tile_pool(name="hbuf", bufs=3) as hbuf,
            tc.tile_pool(name="obuf", bufs=8) as obuf,
            tc.tile_pool(name="preps", bufs=2, space="PSUM") as preps,
            tc.tile_pool(name="mmps", bufs=3, space="PSUM") as mmps,
        ):
            # ---- load params/acts (heT deps first, W2 on SWDGE queue) ----
            we_sb, wd_sb, w2_sb, enc_sb, dec_sb = [], [], [], [], []
            for dc in range(DC):
                t_ = const.tile([P, TSH], bf16, tag=f"enc{dc}")
                nc.sync.dma_start(out=t_[:, :], in_=encT[dc * P:(dc + 1) * P, :])
                enc_sb.append(t_)
                t_ = const.tile([P, H], bf16, tag=f"we{dc}")
                nc.sync.dma_start(out=t_[:, :], in_=w1[dc * P:(dc + 1) * P, :])
                we_sb.append(t_)
            b1_sb = const.tile([P, HC], f32, tag="b1")
            nc.sync.dma_start(out=b1_sb[:, :], in_=b1pc[:, :])
            last_load = None
            for dc in range(DC):
                t_ = const.tile([P, U], bf16, tag=f"dec{dc}")
                nc.sync.dma_start(out=t_[:, :], in_=decT[dc * P:(dc + 1) * P, :])
                dec_sb.append(t_)
                t_ = const.tile([P, H], bf16, tag=f"wd{dc}")
                last_load = nc.sync.dma_start(
                    out=t_[:, :], in_=w1[D + dc * P:D + (dc + 1) * P, :])
                wd_sb.append(t_)
            # ---- heT (+b1) and hdT ----
            heTb_sb, hdT_sb = [], []
            for hc in range(HC):
                ps = preps.tile([P, TSH], mybir.dt.float32, tag="pre")
                for dc in range(DC):
                    nc.tensor.matmul(
                        ps[:, :],
                        we_sb[dc][:, hc * P:(hc + 1) * P],
                        enc_sb[dc][:, :],
                        start=(dc == 0), stop=(dc == DC - 1),
                    )
                t_ = const.tile([P, TSH], f32, tag=f"heTb{hc}")
                nc.vector.tensor_scalar_add(t_[:, :], ps[:, :], b1_sb[:, hc:hc + 1])
                heTb_sb.append(t_)
            for hc in range(HC):
                ps = preps.tile([P, U], mybir.dt.float32, tag="pre")
                for dc in range(DC):
                    nc.tensor.matmul(
                        ps[:, :],
                        wd_sb[dc][:, hc * P:(hc + 1) * P],
                        dec_sb[dc][:, :],
                        start=(dc == 0), stop=(dc == DC - 1),
                    )
                t_ = const.tile([P, U], f32, tag=f"hdT{hc}")
                nc.scalar.copy(t_[:, :], ps[:, :])
                hdT_sb.append(t_)

            # W2 last: not needed until the first main-loop matmul (~8us in),
            # and the serialized DMA path must not delay the preamble loads.
            from concourse.tile_rust import add_dep_helper
            for hc in range(HC):
                t_ = const.tile([P, V], mmdt, tag=f"w2{hc}")
                d_ = nc.gpsimd.dma_start(out=t_[:, :], in_=w2[hc * P:(hc + 1) * P, :])
                add_dep_helper(d_.ins, last_load.ins,
                               reason="defer W2 load behind preamble loads")
                w2_sb.append(t_)

            # ---- main loop over t-blocks ----
            gelu = mybir.ActivationFunctionType.Gelu_apprx_tanh
            # Small first t-blocks shorten the preamble->add->gelu->matmul
            # pipeline-fill chain so PE starts ~2.5us earlier.
            schedule = [4, 4, 4, 4] + [TB] * ((TSH - 16) // TB)
            t0c = 0
            for tlen in schedule:
                h_t = []
                for hc in range(HC):
                    s = sums.tile([P, tlen * U], f32, tag=f"sum{hc}")
                    s3 = s[:, :].rearrange("p (t u) -> p t u", u=U)
                    bc_hd = hdT_sb[hc][:, None, :].broadcast_to((P, tlen, U))
                    bc_he = heTb_sb[hc][:, t0c:t0c + tlen, None].broadcast_to(
                        (P, tlen, U))
                    nc.vector.tensor_tensor(
                        out=s3, in0=bc_hd, in1=bc_he, op=mybir.AluOpType.add)
                    h = hbuf.tile([P, tlen * U], mmdt, tag=f"h{hc}")
                    nc.scalar.activation(h[:, :], s[:, :], gelu)
                    h_t.append(h)
                for blk in range(tlen * U // P):
                    c0 = blk * P
                    p0 = mmps.tile([P, V // 2], mybir.dt.float32, tag="po0")
                    p1 = mmps.tile([P, V // 2], mybir.dt.float32, tag="po1")
                    for hc in range(HC):
                        lhsT = h_t[hc][:, c0:c0 + P]
                        nc.tensor.matmul(
                            p0[:, :], lhsT, w2_sb[hc][:, 0:V // 2],
                            start=(hc == 0), stop=(hc == HC - 1))
                        nc.tensor.matmul(
                            p1[:, :], lhsT, w2_sb[hc][:, V // 2:V],
                            start=(hc == 0), stop=(hc == HC - 1))
                    ob = obuf.tile([P, V], odt, tag="ob")
                    nc.vector.tensor_copy(ob[:, 0:V // 2], p0[:, :])
                    nc.scalar.copy(ob[:, V // 2:V], p1[:, :])
                    row0 = t0c * U + c0
                    nc.sync.dma_start(out=out[row0:row0 + P, :], in_=ob[:, :])
                t0c += tlen
    nc.compile()
    return nc


def _get_nc(mm_bf16=True):
    key = mm_bf16
    if key not in _NC_CACHE:
        _NC_CACHE[key] = _build_module(mm_bf16)
    return _NC_CACHE[key]


def kernel(encoder_outputs, decoder_outputs, W1, b1, W2):
    global LAST_RESULT
    from concourse.bass_utils import run_bass_kernel_spmd

    bfl = ml_dtypes.bfloat16
    enc = np.ascontiguousarray(np.asarray(encoder_outputs, dtype=np.float32).astype(bfl))
    dec = np.ascontiguousarray(np.asarray(decoder_outputs, dtype=np.float32).astype(bfl))
    w1 = np.ascontiguousarray(np.asarray(W1, dtype=np.float32).astype(bfl))
    b1v = np.asarray(b1, dtype=np.float32)
    w2 = np.asarray(W2, dtype=np.float32)

    mm_bf16 = True
    w2c = np.ascontiguousarray(w2.astype(bfl) if mm_bf16 else w2)
    b1pc = np.ascontiguousarray(b1v.reshape(HC, P).T)   # [128, 4]

    nc = _get_nc(mm_bf16)   # key covers out dtype too (module-level OUT_BF16)
    in_maps = []
    for k in range(NCORES):
        b = k // (T // TSH)
        t0 = (k % (T // TSH)) * TSH
        in_maps.append({
            "encT": np.ascontiguousarray(enc[b, t0:t0 + TSH, :].T),
            "decT": np.ascontiguousarray(dec[b].T),
            "W1": w1,
            "b1pc": b1pc,
            "W2c": w2c,
        })

    res = run_bass_kernel_spmd(
        nc, in_maps, core_ids=list(range(NCORES)), trace=TRACE)
    LAST_RESULT = res
    out = np.empty((B, T, U, V), dtype=np.float32)
    for k in range(NCORES):
        b = k // (T // TSH)
        t0 = (k % (T // TSH)) * TSH
        shard = res.results[k]["out"].reshape(TSH, U, V)
        out[b, t0:t0 + TSH] = shard.astype(np.float32)
    return out

